# revision 3
# baseline (speedup 1.0000x reference)
"""Trainium2 Bass kernel for ContextQueryAttention (BiDAF-style trilinear attention).

Computes, per batch n:
    sim[c,q] = <ctx[c], wc> + <xq[q], wc> + <ctx[c] * wcq, xq[q]>
    c2q  = softmax_q(sim) @ xq                      # [C, F]
    q2c  = softmax_c(max_q sim) @ ctx               # [F]
    out  = concat([ctx, c2q, ctx*c2q, ctx*q2c], -1) # [C, 4F]

Sharding: data-parallel over batch N=64 across 8 NeuronCores (8 batches/core).

Per-core structure (per batch):
  - all PE matmuls in bf16 (fp32 runs at 4 cyc/row vs bf16 1 cyc/row); PSUM
    accumulation stays fp32 so logits keep ~0.05 abs error on std-32 rows --
    softmax argmax is stable and l2 err stays ~1e-3, far under the 2e-2 gate
  - ctx cast to bf16 (ctxb) for PE use; fp32 ctx kept for term1/3/4 and stores
  - ctxT built via 32 bf16 PE transposes (f-contraction needs f-major operands)
  - sim psum [128c, 129] per c-tile: 4 K-chunk matmuls with an augmented
    moving operand [wcq*xqT | wc] so column 128 accumulates s_ctx for free,
    plus a rank-1 (ones x s_qry) matmul for the query bias term
  - softmax over q on the free axis: DVE reduce_max(negate) -> ACT exp with
    per-partition bias, bf16 E out, fp32 accumulated row-sum
  - q2c chain (gpsimd cross-partition max) issued after 2 c2q tiles so the
    PE keeps streaming during the all-reduce latency
  - pass 2 per tile: E^T -> c2q matmul -> normalize / term3 / term4 -> ONE
    [128, 1536] store per tile, so output DMA flows through the whole batch
    instead of bursting at batch end
  - ctx (term1) stored as one merged DMA per batch during pass 1
  - loads ride the ACT HWDGE ring; stores the SP ring
"""

import os

os.environ.setdefault("JAX_PLATFORMS", "axon")

import numpy as np

import concourse.bass as bass
import concourse.mybir as mybir
import concourse.tile as tile
from concourse import bacc, bass_isa, bass_utils
from concourse.masks import make_identity

f32 = mybir.dt.float32
bf16 = mybir.dt.bfloat16
AX = mybir.AxisListType.X
EXP = mybir.ActivationFunctionType.Exp
COPY = mybir.ActivationFunctionType.Copy
MULT = mybir.AluOpType.mult
ADD = mybir.AluOpType.add

N_CORES = 8
B = 8          # batches per core
C = 1024       # context length
Q = 128        # query length
F = 512        # feature dim
CT = C // 128  # c-tiles per batch
FC = F // 128  # f-chunks


def build_nc():
    nc = bacc.Bacc("TRN2", target_bir_lowering=False, debug=False)
    xc = nc.dram_tensor("x_context", [B, C, F], f32, kind="ExternalInput").ap()
    xq_d = nc.dram_tensor("x_query", [B, Q, F], f32, kind="ExternalInput").ap()
    wc_d = nc.dram_tensor("w_context", [F], f32, kind="ExternalInput").ap()
    wcq_d = nc.dram_tensor("w_cq", [F], f32, kind="ExternalInput").ap()
    out = nc.dram_tensor("out", [B, C, 4 * F], f32, kind="ExternalOutput").ap()

    from contextlib import ExitStack

    with tile.TileContext(nc) as tc, ExitStack() as es:
        def pool(name, bufs, space="SBUF"):
            return es.enter_context(tc.tile_pool(name=name, bufs=bufs, space=space))

        const = pool("const", 1)
        ctx_p = pool("ctx_p", 2)
        ctxb_p = pool("ctxb_p", 2)
        ctxT_p = pool("ctxT_p", 2)
        xq_p = pool("xq_p", 2)
        xqb_p = pool("xqb_p", 2)
        xqw_p = pool("xqw_p", 2)
        tmp_p = pool("tmp_p", 2)
        e_p = pool("e_p", CT + 2)
        et_p = pool("et_p", 3)
        asm_p = pool("asm_p", 4)
        vec_p = pool("vec_p", CT + 2)
        sml_p = pool("sml_p", 2)
        ps_sim_p = pool("ps_sim", 2, "PSUM")
        ps_ctxT_p = pool("ps_ctxT", 2, "PSUM")
        ps_c2q_p = pool("ps_c2q", 2, "PSUM")
        ps_sml_p = pool("ps_sml", 2, "PSUM")

        # loads on the ACT HWDGE ring; stores on the SP ring
        dma_load = nc.scalar.dma_start
        dma_store = nc.sync.dma_start

        ident = const.tile([128, 128], f32)
        make_identity(nc, ident)
        identb = const.tile([128, 128], bf16)
        nc.vector.tensor_copy(identb, ident)
        ones_rowb = const.tile([1, 128], bf16)
        nc.vector.memset(ones_rowb, 1.0)
        ones_col = const.tile([128, 1], f32)
        nc.vector.memset(ones_col, 1.0)
        wc_sb = const.tile([128, FC], f32)
        dma_load(wc_sb, wc_d.rearrange("(a p) -> p a", p=128))
        wc_sbb = const.tile([128, FC], bf16)
        nc.vector.tensor_copy(wc_sbb, wc_sb)
        wcq_sb = const.tile([128, FC], f32)
        dma_load(wcq_sb, wcq_d.rearrange("(a p) -> p a", p=128))
        wc_row = const.tile([1, F], f32)
        dma_load(wc_row, wc_d[None, :])
        wc_rowb = const.tile([1, F], bf16)
        nc.vector.tensor_copy(wc_rowb, wc_row)
        # wc broadcast along partitions (for s_qry): ones[1,128]^T @ wc[1,512]
        ps_wcb = ps_sml_p.tile([128, F], f32, tag="sml")
        nc.tensor.matmul(ps_wcb, lhsT=ones_rowb, rhs=wc_rowb, start=True, stop=True)
        wc_bc = const.tile([128, F], f32)
        nc.vector.tensor_copy(wc_bc, ps_wcb)

        def load_batch(b):
            ctx = ctx_p.tile([128, CT, F], f32, name="ctx")
            dma_load(ctx, xc[b].rearrange("(t p) f -> p t f", p=128))
            xq = xq_p.tile([128, F], f32, name="xq")
            dma_load(xq, xq_d[b])
            return ctx, xq

        nxt = load_batch(0)
        for b in range(B):
            # ---- loads (prefetched one batch ahead) ----
            ctx, xq = nxt
            if b + 1 < B:
                nxt = load_batch(b + 1)

            # ---- term1: merged ctx store, flows during pass 1 ----
            dma_store(out[b, :, 0:F].rearrange("(t p) f -> p t f", p=128), ctx)

            # ---- xq bf16 copy (c2q rhs + xqT source) ----
            xqb = xqb_p.tile([128, F], bf16, name="xqb")
            nc.vector.tensor_copy(xqb, xq)

            # ---- xqT, scaled by w_cq, augmented with wc column ----
            # xqw_aug[:, fc] = [wcq*xqT chunk | wc chunk]   ([128, 129] bf16)
            xqw_aug = xqw_p.tile([128, FC, Q + 1], bf16)
            for fc in range(FC):
                ps_xqT = ps_sml_p.tile([128, 128], bf16, tag="sml")
                nc.tensor.transpose(ps_xqT, xqb[:, fc * 128 : (fc + 1) * 128], identb)
                nc.scalar.activation(
                    xqw_aug[:, fc, 0:Q], ps_xqT, COPY,
                    scale=wcq_sb[:, fc : fc + 1],
                )
                nc.vector.tensor_copy(
                    xqw_aug[:, fc, Q : Q + 1], wc_sbb[:, fc : fc + 1]
                )

            # ---- s_qry row [1, 128] (fused mul+reduce, then PE transpose) ----
            scr = tmp_p.tile([128, F], f32, name="scr", tag="scr")
            sq_col = vec_p.tile([128, 1], f32, tag="sqcol")
            nc.vector.tensor_mul(scr, xq, wc_bc)
            nc.vector.reduce_sum(sq_col, scr, axis=AX)
            ps_sqT = ps_sml_p.tile([1, 128], f32, tag="sml")
            nc.tensor.transpose(ps_sqT, sq_col, ident)
            sq_rowb = sml_p.tile([1, 128], bf16, name="sq_rowb", tag="sq_row")
            nc.scalar.copy(sq_rowb, ps_sqT)

            # ---- ctxb (bf16 cast) + ctxT [f, c], interleaved with pass 1 ----
            ctxb = ctxb_p.tile([128, CT, F], bf16, name="ctxb")
            ctxT = ctxT_p.tile([128, FC, C], bf16)

            cast_fns = [
                nc.vector.tensor_copy,
                nc.gpsimd.tensor_copy,
                nc.scalar.copy,
                nc.vector.tensor_copy,
            ]

            def stage_ctxT(half):
                for j in range(4):
                    t = half * 4 + j
                    cast_fns[j % len(cast_fns)](ctxb[:, t], ctx[:, t])
                for fc in range(FC):
                    ps_ct = ps_ctxT_p.tile([128, 512], bf16)
                    for j in range(4):
                        t = half * 4 + j
                        nc.tensor.transpose(
                            ps_ct[:, j * 128 : (j + 1) * 128],
                            ctxb[:, t, fc * 128 : (fc + 1) * 128],
                            identb,
                        )
                    cp = nc.vector.tensor_copy if fc % 2 == 0 else nc.scalar.copy
                    cp(ctxT[:, fc, half * 512 : (half + 1) * 512], ps_ct)

            # ---- pass 1: sim + softmax stats per c-tile ----
            z = sml_p.tile([128, CT], f32, name="z", tag="z")
            Es = []
            rcps = []

            def pass1_tile(t):
                ps_sim = ps_sim_p.tile([128, Q + 1], f32)
                for fc in range(FC):
                    nc.tensor.matmul(
                        ps_sim,
                        lhsT=ctxT[:, fc, t * 128 : t * 128 + 128],
                        rhs=xqw_aug[:, fc],
                        start=(fc == 0),
                        stop=False,
                    )
                nc.tensor.matmul(
                    ps_sim[:, 0:Q], lhsT=ones_rowb, rhs=sq_rowb, start=False, stop=True
                )
                nmax = vec_p.tile([128, 1], f32, tag="nmax")
                nc.vector.reduce_max(nmax, ps_sim[:, 0:Q], axis=AX, negate=True)
                E = e_p.tile([128, Q], bf16)
                rsum = vec_p.tile([128, 1], f32, tag="rsum")
                nc.scalar.activation(E, ps_sim[:, 0:Q], EXP, bias=nmax, accum_out=rsum)
                rcp = vec_p.tile([128, 1], f32, tag="rcp")
                nc.vector.reciprocal(rcp, rsum)
                # z[:, t] = s_ctx + rowmax = psum[:,128] - (-max)
                nc.vector.tensor_sub(z[:, t : t + 1], ps_sim[:, Q : Q + 1], nmax)
                Es.append(E)
                rcps.append(rcp)

            stage_ctxT(0)
            for t in range(4):
                pass1_tile(t)
            stage_ctxT(1)
            for t in range(4, CT):
                pass1_tile(t)

            # ---- q2c softmax prep (issued later, overlapped with pass 2) ----
            def q2c_prep():
                zmax = vec_p.tile([128, 1], f32, tag="zmax")
                nc.vector.reduce_max(zmax, z, axis=AX)
                gmax = vec_p.tile([128, 1], f32, tag="gmax")
                nc.gpsimd.partition_all_reduce(
                    gmax, zmax, channels=128, reduce_op=bass_isa.ReduceOp.max
                )
                negb = vec_p.tile([128, 1], f32, tag="negb")
                nc.vector.tensor_scalar_mul(negb, gmax, -1.0)
                expz = sml_p.tile([128, CT], bf16, name="expz", tag="expz")
                ers = vec_p.tile([128, 1], f32, tag="ers")
                nc.scalar.activation(expz, z, EXP, bias=negb, accum_out=ers)
                return expz, ers

            def q2c_matmuls(expz, ers):
                ps_S = ps_sml_p.tile([1, 1], f32, tag="sml")
                nc.tensor.matmul(ps_S, lhsT=ers, rhs=ones_col, start=True, stop=True)
                rS = sml_p.tile([1, 1], f32, name="rS", tag="rS")
                nc.vector.reciprocal(rS, ps_S)
                ps_q2c = ps_sml_p.tile([1, F], f32, tag="sml")
                for t in range(CT):
                    nc.tensor.matmul(
                        ps_q2c,
                        lhsT=expz[:, t : t + 1],
                        rhs=ctxb[:, t],
                        start=(t == 0),
                        stop=(t == CT - 1),
                    )
                xq2cb_ = sml_p.tile([1, F], bf16, name="xq2c", tag="xq2c")
                nc.scalar.activation(xq2cb_, ps_q2c, COPY, scale=rS)
                ps_bc = ps_sml_p.tile([128, F], f32, tag="sml")
                nc.tensor.matmul(ps_bc, lhsT=ones_rowb, rhs=xq2cb_, start=True, stop=True)
                xq2cb = tmp_p.tile([128, F], f32, name="xq2cb", tag="xq2cb")
                nc.vector.tensor_copy(xq2cb, ps_bc)
                return xq2cb

            # ---- pass 2 (software-pipelined): E^T one tile ahead of c2q ----
            def stage_et(t):
                ps_et = ps_sml_p.tile([128, Q], bf16, tag="sml")
                nc.tensor.transpose(ps_et, Es[t], identb)
                ET = et_p.tile([128, Q], bf16)
                nc.scalar.copy(ET, ps_et)
                return ET

            def stage_c2q(t, ET):
                ps_c2q = ps_c2q_p.tile([128, F], f32)
                nc.tensor.matmul(ps_c2q, lhsT=ET, rhs=xqb, start=True, stop=True)
                asm = asm_p.tile([128, 3 * F], f32)
                # normalized c2q, fused into the psum->sbuf move
                if t % 2 == 0:
                    nc.scalar.activation(asm[:, 0:F], ps_c2q, COPY, scale=rcps[t])
                else:
                    nc.vector.tensor_scalar_mul(asm[:, 0:F], ps_c2q, rcps[t])
                nc.vector.tensor_mul(asm[:, F : 2 * F], ctx[:, t], asm[:, 0:F])
                return asm

            def stage_term4_store(t, asm, xq2cb):
                eng = nc.vector if t % 4 == 0 else nc.gpsimd
                eng.tensor_mul(asm[:, 2 * F : 3 * F], ctx[:, t], xq2cb)
                dma_store(out[b, t * 128 : (t + 1) * 128, F : 4 * F], asm)

            expz, ers = q2c_prep()
            # run 2 c2q tiles first so the PE streams during the all-reduce
            ET0 = stage_et(0)
            ET1 = stage_et(1)
            asm0 = stage_c2q(0, ET0)
            asm1 = stage_c2q(1, ET1)
            xq2cb = q2c_matmuls(expz, ers)
            stage_term4_store(0, asm0, xq2cb)
            stage_term4_store(1, asm1, xq2cb)
            prev = None
            for t in range(2, CT):
                ET = stage_et(t)
                if prev is not None:
                    asm = stage_c2q(prev[0], prev[1])
                    stage_term4_store(prev[0], asm, xq2cb)
                prev = (t, ET)
            asm = stage_c2q(prev[0], prev[1])
            stage_term4_store(prev[0], asm, xq2cb)

    nc.compile()
    return nc


_NC = None


def kernel(**inputs):
    global _NC
    if _NC is None:
        _NC = build_nc()
    xc = np.ascontiguousarray(np.asarray(inputs["x_context"], dtype=np.float32))
    xq = np.ascontiguousarray(np.asarray(inputs["x_query"], dtype=np.float32))
    wc = np.ascontiguousarray(np.asarray(inputs["w_context"], dtype=np.float32))
    wcq = np.ascontiguousarray(np.asarray(inputs["w_cq"], dtype=np.float32))
    in_maps = [
        {
            "x_context": xc[i * B : (i + 1) * B],
            "x_query": xq[i * B : (i + 1) * B],
            "w_context": wc,
            "w_cq": wcq,
        }
        for i in range(N_CORES)
    ]
    res = bass_utils.run_bass_kernel_spmd(_NC, in_maps, core_ids=list(range(N_CORES)))
    return np.concatenate([res.results[i]["out"] for i in range(N_CORES)], axis=0)
